# revision 19
# baseline (speedup 1.0000x reference)
"""Trainium2 Bass kernel for nn_LongRangeModule (gnn_message_passing).

Strategy (sequence-parallel over i, mask-compacted, fp8 DoubleRow):
  - Host: select masked-in rows (compaction), normalize embeddings and scale
    by 8 -> fp8 e4m3 (cos8 = 64*cos), pack j-operands in DoubleRow pair
    layout [pair, 128, k2, *], shard i-rows over 8 cores (640 rows each,
    5x128 subtiles in windows [256, 256, 128]).
  - j-blocks are rotated per core (by an even block count) so the near-band
    blocks (|pos_i - pos_j| <= 128 possible) sit at fixed LOCAL indices:
    6 slots per window get a far-mask strip; all other blocks are provably
    all-far and skip the strip entirely.
  - Device per core, per window, per j-block-pair t:
      cos8[j,i] = DoubleRow MM(nrmj8[t,q], nrmi8[:, :, win])      (PE, fp8)
      absc = |cos8| -> bf16                                        (ACT)
      src  = absc * strip   (near slots only)                      (DVE)
      wt8[:, q, :] = (src > 6.4) * src -> fp8                      (DVE, fused)
      m01pm = sign(src - 6.4) -> bf16 in {-1, +1}                  (ACT)
      agg[s,b] += DoubleRow MM(wt8[:, :, s], xj8[t][:, :, b])      (PE, fp8)
      njpm[:, s] += MM(m01pm[:, s], ones)   (N=1 matmuls, accumulated
        with start=False into a DVE-zeroed PSUM bank so the per-s groups
        can interleave without clearing each other's has_written bits)
    Window end: num_j = (njpm + NJ)/2  (exact; pad rows count as -1)
    Epilogue: y = t*x + sc*agg, t = 1-0.5*z, sc = (0.5/64)*z/max(nj,1),
    z = nj>0; y written bf16, upcast+scattered on host.
"""

import math
import sys

import numpy as np

try:
    import concourse.bass as bass
except ImportError:  # harness env may not have the repo on sys.path
    sys.path.insert(0, "/opt/trn_rl_repo")
    import concourse.bass as bass

import ml_dtypes
import concourse.mybir as mybir
from concourse.bass_utils import run_bass_kernel_spmd
from concourse.tile import TileContext

BF16 = ml_dtypes.bfloat16
E4 = ml_dtypes.float8_e4m3
F32 = mybir.dt.float32
BF = mybir.dt.bfloat16
F8 = mybir.dt.float8e4
AF = mybir.ActivationFunctionType
OP = mybir.AluOpType
DR = mybir.MatmulPerfMode.DoubleRow

B, L, D, E = 2, 8192, 512, 256
CHUNK, CUT, EPS = 128, 0.1, 1e-8
NCORES = 8
SCALE = 8.0  # nrm scale; cos8 = SCALE^2 * cos
CUT8 = CUT * SCALE * SCALE
ASCALE = 1.0 / (SCALE * SCALE)
NSLOT = 6  # near-band strip slots per window

TRACE = False
LAST = {}  # stash for test.py (exec_time_ns etc.)


def _plan(N):
    chunk = -(-N // NCORES)  # i-rows each core is responsible for
    nsub = -(-chunk // 128)  # 128-row subtiles per core
    per = nsub * 128
    windows = [256] * (nsub // 2) + ([128] if nsub % 2 else [])
    NJB = -(-N // 128)
    NJBp = NJB + (NJB & 1)
    NJP2 = NJBp // 2
    return chunk, nsub, per, windows, NJB, NJBp, NJP2


def _slots(windows):
    """[(iw, local_block, slot_index)] for near-band strips."""
    out = []
    k = 0
    ibs = 0
    for iw, w in enumerate(windows):
        for j in range(NSLOT):
            out.append((iw, ibs - 1 + j, k))
            k += 1
        ibs += w // 128
    return out


def _build(nc: bass.Bass, N: int):
    chunk, nsub, per, windows, NJB, NJBp, NJP2 = _plan(N)
    BD = B * D
    slotmap = {(iw, lb): k for iw, lb, k in _slots(windows)}

    nrmj = nc.dram_tensor("nrmj", [NJP2, 128, 4, 128], F8, kind="ExternalInput")
    nrmi = nc.dram_tensor("nrmi", [128, 2, per], F8, kind="ExternalInput")
    xj = nc.dram_tensor("xj", [NJP2, 128, 2, BD], F8, kind="ExternalInput")
    xi = nc.dram_tensor("xi", [nsub, B, 128, D], BF, kind="ExternalInput")
    strips = nc.dram_tensor(
        "strips", [NSLOT * len(windows), 128, 256], BF, kind="ExternalInput"
    )
    y = nc.dram_tensor("y", [nsub, B, 128, D], BF, kind="ExternalOutput")

    with (
        TileContext(nc) as tc,
        tc.tile_pool(name="res", bufs=1) as res,
        tc.tile_pool(name="stp", bufs=3) as stp,
        tc.tile_pool(name="wk", bufs=4) as wk,
        tc.tile_pool(name="wt", bufs=3) as wtp,
        tc.tile_pool(name="epi", bufs=3) as ep,
        tc.tile_pool(name="pcos", bufs=2, space="PSUM") as pcos,
        tc.tile_pool(name="pacc", bufs=1, space="PSUM") as pacc,
    ):
        # resident operands (small first so compute can start early)
        nrmi_sb = res.tile([128, 2, per], F8, tag="nrmi_sb")
        nc.sync.dma_start(out=nrmi_sb[:], in_=nrmi[:])
        ones_col = res.tile([128, 1], BF, tag="ones_col")
        nc.vector.memset(ones_col[:], 1.0)
        one_one = res.tile([1, 1], F32, tag="one_one")
        nc.vector.memset(one_one[:], 1.0)
        # per-window pair order: strip-free (far) pairs first so the strip
        # DMAs are never on the critical path of the PSUM accumulation chain
        def near(iw, lb):
            k = slotmap.get((iw, lb))
            if k is None:  # wrap: local block NJBp-1 is slot -1 of iw 0
                k = slotmap.get((iw, lb - NJBp))
            return k

        orders = []
        for iw in range(len(windows)):
            ts = list(range(NJP2))
            ts.sort(key=lambda t: (near(iw, 2 * t) is not None)
                    or (near(iw, 2 * t + 1) is not None))
            orders.append(ts)

        nrmj_sb = [None] * NJP2
        xj_sb = [None] * NJP2
        for t in orders[0]:  # load in first-use order
            nj = res.tile([128, 4, 128], F8, tag=f"nrmj{t}", name=f"nrmj{t}")
            nc.sync.dma_start(out=nj[:], in_=nrmj[t])
            nrmj_sb[t] = nj
        for t in orders[0]:
            xt = res.tile([128, 2, BD], F8, tag=f"xj{t}", name=f"xj{t}")
            nc.sync.dma_start(out=xt[:], in_=xj[t])
            xj_sb[t] = xt

        NJtot = float(NJBp * 128)
        ibs = 0  # window's first subtile index
        for iw, W in enumerate(windows):
            nsw = W // 128
            lo = ibs * 128
            aggs = [
                pacc.tile([128, D], F32, tag=f"agg{k}", name=f"agg{k}")
                for k in range(nsw * B)
            ]
            njp = pacc.tile([128, 512], F32, tag="njp")
            njrow = pacc.tile([1, 512], F32, tag="njrow")
            nc.vector.memset(njrow[:], 0.0)
            order = orders[iw]
            for ti, t in enumerate(order):
                first, last = ti == 0, ti == NJP2 - 1
                wt8 = wtp.tile([128, 2, W], F8, tag="wt8")
                abst = wk.tile([128, 2, W], BF, tag="absc2", name="absc2")
                srcs = []
                anynear = False
                for q in (0, 1):
                    lb = 2 * t + q
                    cos = pcos.tile([128, 512], F32, tag="cos")
                    nc.tensor.matmul(
                        cos[:, :W],
                        nrmj_sb[t][:, 2 * q : 2 * q + 2, :],
                        nrmi_sb[:, :, lo : lo + W],
                        start=True,
                        stop=True,
                        perf_mode=DR,
                    )
                    nc.scalar.activation(abst[:, q, :], cos[:, :W], AF.Abs)
                    src = abst[:, q, :]
                    k = near(iw, lb)
                    if k is not None:
                        anynear = True
                        strip = stp.tile([128, W], BF, tag="strip")
                        nc.sync.dma_start(out=strip[:], in_=strips[k][:, :W])
                        am = wk.tile([128, W], BF, tag="am")
                        nc.vector.tensor_mul(am[:], abst[:, q, :], strip[:])
                        src = am[:]
                    nc.vector.scalar_tensor_tensor(
                        wt8[:, q, :], src, CUT8, src, op0=OP.is_gt, op1=OP.mult
                    )
                    srcs.append(src)
                # (src > CUT8)*2 -> {0, 2}; one batched op for far pairs
                m01 = wk.tile([128, 2, W], BF, tag="m01", name="m01")
                if anynear:
                    for q in (0, 1):
                        nc.vector.tensor_scalar(
                            m01[:, q, :], srcs[q], CUT8, 2.0, op0=OP.is_gt, op1=OP.mult
                        )
                else:
                    nc.vector.tensor_scalar(
                        m01[:], abst[:], CUT8, 2.0, op0=OP.is_gt, op1=OP.mult
                    )
                for s in range(nsw):
                    for b in range(B):
                        nc.tensor.matmul(
                            aggs[s * B + b][:],
                            wt8[:, :, s * 128 : (s + 1) * 128],
                            xj_sb[t][:, :, b * D : (b + 1) * D],
                            start=first,
                            stop=last,
                            perf_mode=DR,
                        )
                # row-sums over j: njrow[0, q*W + i] += sum_p m01[p, q, i]
                nc.tensor.matmul(
                    njrow[0:1, 0 : 2 * W],
                    ones_col[:],
                    m01[:],
                    start=False,
                    stop=last,
                    skip_group_check=True,
                )
            # fold q + transpose njrow -> per-partition njp columns via K=1
            # matmuls (njp = 2 * num_j)
            njrow_sb = ep.tile([1, 512], F32, tag="njrow_sb", name="njrow_sb")
            nc.scalar.activation(njrow_sb[0:1, 0 : 2 * W], njrow[0:1, 0 : 2 * W], AF.Copy)
            for s in range(nsw):
                for q in (0, 1):
                    nc.tensor.matmul(
                        njp[:, s : s + 1],
                        njrow_sb[0:1, q * W + s * 128 : q * W + (s + 1) * 128],
                        one_one[:],
                        start=(q == 0),
                        stop=(q == 1),
                    )
            # epilogue
            for s in range(nsw):
                nj = ep.tile([128, 1], F32, tag="nj")
                nc.vector.tensor_scalar(
                    nj[:], njp[:, s : s + 1], 0.5, None, op0=OP.mult
                )
                z = ep.tile([128, 1], F32, tag="z")
                nc.vector.tensor_scalar(z[:], nj[:], 0.0, None, op0=OP.is_gt)
                mx = ep.tile([128, 1], F32, tag="mx")
                nc.vector.tensor_scalar(mx[:], nj[:], 1.0, None, op0=OP.max)
                r = ep.tile([128, 1], F32, tag="r")
                nc.vector.reciprocal(r[:], mx[:])
                sc0 = ep.tile([128, 1], F32, tag="sc0")
                nc.vector.tensor_scalar(sc0[:], r[:], 0.5 * ASCALE, None, op0=OP.mult)
                sc = ep.tile([128, 1], F32, tag="sc")
                nc.vector.tensor_mul(sc[:], sc0[:], z[:])
                tt = ep.tile([128, 1], F32, tag="tt")
                nc.vector.tensor_scalar(tt[:], z[:], -0.5, 1.0, op0=OP.mult, op1=OP.add)
                for b in range(B):
                    xis = ep.tile([128, D], BF, tag="xis")
                    nc.sync.dma_start(out=xis[:], in_=xi[ibs + s, b])
                    ag = ep.tile([128, D], F32, tag="ag")
                    nc.scalar.activation(
                        ag[:], aggs[s * B + b][:], AF.Copy, bias=0.0, scale=sc[:]
                    )
                    yt = ep.tile([128, D], BF, tag="yt")
                    nc.vector.scalar_tensor_tensor(
                        yt[:], xis[:], tt[:], ag[:], op0=OP.mult, op1=OP.add
                    )
                    nc.sync.dma_start(out=y[ibs + s, b], in_=yt[:])
            ibs += nsw
    return nc


_NOSPLIT = ("InstEventSemaphore", "InstAllEngineBarrier")


def _split_waits(nc):
    """This walrus rejects >1 sync wait on TPB compute instructions; hoist
    extra waits onto per-wait EventSemaphore instructions just before."""
    nev = 0
    for f in nc.m.functions:
        for bb in f.blocks:
            out = []
            changed = False
            for inst in bb.instructions:
                si = getattr(inst, "sync_info", None)
                ow = list(si.on_wait) if si and si.on_wait else []
                if len(ow) >= 2 and type(inst).__name__ not in _NOSPLIT:
                    for w in ow[:-1]:
                        nev += 1
                        out.append(
                            mybir.InstEventSemaphore(
                                name=f"EVW-{nev}",
                                engine=inst.engine,
                                ins=[],
                                outs=[],
                                sync_info=mybir.SyncInfo(on_wait=[w], on_update=[]),
                            )
                        )
                    inst.sync_info = mybir.SyncInfo(
                        on_wait=ow[-1:], on_update=list(si.on_update or [])
                    )
                    changed = True
                out.append(inst)
            if changed:
                bb.instructions = out


def _host_prep(x, mask, emb_i, emb_j):
    m = mask.astype(bool)
    idx = np.where(m)[0]
    N = len(idx)
    assert N > 0
    chunk, nsub, per, windows, NJB, NJBp, NJP2 = _plan(N)
    BD = B * D

    def nrm(e):
        n = np.maximum(np.linalg.norm(e, axis=-1, keepdims=True), EPS)
        return (e / n * SCALE).astype(np.float32)

    ni8 = nrm(emb_i).astype(E4).astype(np.float32)  # keep f32 copy for emul
    nj8 = nrm(emb_j).astype(E4)

    NJ = NJBp * 128
    # j operands (global, block-pair DoubleRow layout)
    njp_rows = np.zeros((NJ, E), E4)
    njp_rows[:N] = nj8[idx]
    # [t, q, jj, k, p] -> [t, p, q, k, jj] -> [NJP2, 128, 4, 128]
    tmp = njp_rows.reshape(NJP2, 2, 128, 2, 128)  # [t, q, jj, k, p]
    nrmj_h = np.ascontiguousarray(tmp.transpose(0, 4, 1, 3, 2)).reshape(
        NJP2, 128, 4, 128
    )
    xsel = np.zeros((NJ, BD), np.float32)
    xsel[:N] = np.transpose(x[:, idx], (1, 0, 2)).reshape(N, BD)
    x8 = xsel.astype(E4)
    # [t, k, p, bd] -> [t, p, k, bd]
    xj_h = np.ascontiguousarray(
        x8.reshape(NJP2, 2, 128, BD).transpose(0, 2, 1, 3)
    )
    pj = np.full(NJ, -(10**6), np.int64)
    pj[:N] = idx

    slots = _slots(windows)
    in_maps = []
    meta = []
    for c in range(NCORES):
        s_c = min(c * chunk, N - 1)
        rows = np.clip(s_c + np.arange(per), 0, N - 1)
        gi = idx[rows]
        # nrmi [p, k, i]
        nis = ni8[gi].astype(E4)  # (per, E)
        nrmi_h = np.ascontiguousarray(nis.reshape(per, 2, 128).transpose(2, 1, 0))
        xi_h = np.ascontiguousarray(
            np.transpose(x[:, gi].reshape(B, nsub, 128, D), (1, 0, 2, 3))
        ).astype(BF16)
        # rotation (even block count so DR pairs stay aligned)
        r_c = 2 * (s_c // 256)
        pperm = (r_c // 2 + np.arange(NJP2)) % NJP2
        strips_h = np.ones((len(slots), 128, 256), BF16)
        ibs = 0
        for iw, W in enumerate(windows):
            pi = pj[:N][rows[ibs * 128 : ibs * 128 + W]]  # orig positions (real rows)
            pi = idx[rows[ibs * 128 : ibs * 128 + W]]
            for jw, lb, k in slots:
                if jw != iw:
                    continue
                g = (r_c + lb) % NJBp
                pjj = pj[g * 128 : (g + 1) * 128]
                dmat = np.abs(pjj[:, None] - pi[None, :])
                strips_h[k, :, :W] = (dmat > CHUNK).astype(BF16)
            ibs += W // 128
        in_maps.append(
            {
                "nrmj": nrmj_h[pperm],
                "nrmi": nrmi_h,
                "xj": xj_h[pperm],
                "xi": xi_h,
                "strips": strips_h,
            }
        )
        meta.append((s_c, min(N - s_c, chunk)))
    return in_maps, idx, N, meta


def kernel(x, mask, emb_i, emb_j):
    x = np.asarray(x, np.float32)
    mask = np.asarray(mask)
    emb_i = np.asarray(emb_i, np.float32)
    emb_j = np.asarray(emb_j, np.float32)

    in_maps, idx, N, meta = _host_prep(x, mask, emb_i, emb_j)
    chunk, nsub, per, windows, NJB, NJBp, NJP2 = _plan(N)
    nc = bass.Bass()
    _build(nc, N)
    _split_waits(nc)
    res = run_bass_kernel_spmd(nc, in_maps, list(range(NCORES)), trace=TRACE)
    LAST["res"] = res
    out = x.copy()
    for c in range(NCORES):
        s_c, cnt = meta[c]
        yc = res.results[c]["y"].astype(np.float32)  # [nsub, B, 128, D]
        yr = np.transpose(yc, (1, 0, 2, 3)).reshape(B, per, D)
        out[:, idx[s_c : s_c + cnt]] = yr[:, :cnt]
    return out
